# Initial kernel scaffold
#
"""Multi-Head Latent Attention on 8 trn2 NeuronCores (Bass/Tile).

Sharding: core c = 4*b + g handles batch b, head group g (4 of 16 heads).
Host absorbs the latent down/up projections (W_k = W_kc @ W_ku etc.), folds
the 1/sqrt(d_k) scale into W_q, pre-transposes q/k/v to [D, S], and
pre-rounds all matmul inputs to fp32r (the PE's fast 4-byte format; exact
on pre-rounded data).

Per-core device pipeline (S=2048, D=2048, d_k=128, 4 heads, e=4*128=512):
  A. QT[h] [128,2048] = (W_q_h.T q.T)      (d-outer accumulate, N=512)
  B. KT[h] [128,2048] = (W_k_h.T k.T)
  C. V    [S,512] as 4 tiles [128,2048]    (v.T stationary, W_v moving)
  D. pass1 per (h, sq-tile): scores = QT_h.T KT_h -> exp (accum row sums)
     -> attn = E * (1/sum) -> HBM; also ln(sum) transposed into a row
  E. pass2 per (h, sq-chunk): scoresT = KT_h.T QT_h; add -ln(sum) row via
     K=1 ones-matmul; exp -> normalized attn^T (fp32r); AV accumulate
     outT[h] [128 e, 2048 sq]
  F. O-proj: out[sq, dm] = sum_h OT_h.T W_o_h -> HBM (normalized already)
Host: attn = stack of per-core [4,S,S]; output[b] = sum of 4 cores' outp.
"""
import numpy as np
import concourse.bass as bass
import concourse.mybir as mybir
from concourse import bacc, tile
from concourse.bass_utils import run_bass_kernel_spmd

AF = mybir.ActivationFunctionType
DT = mybir.dt.float32
DTR = mybir.dt.float32r
AX = mybir.AxisListType

B, S_FULL, D, H, DK, L = 2, 2048, 2048, 16, 128, 512
N_CORES = 8
HC = 4          # heads per core
EC = HC * DK    # 512


def round_fp32r(x):
    x = np.ascontiguousarray(x, dtype=np.float32)
    u = x.view(np.uint32).astype(np.uint64)
    u = (u + 0x7FF + ((u >> 12) & 1)) & 0xFFFFF000
    return u.astype(np.uint32).view(np.float32)


def build_nc(reps=1, S=S_FULL):
    NSQ = S // 128   # sq tiles
    NSC = S // 512   # 512-wide chunks
    nc = bacc.Bacc("TRN2", target_bir_lowering=False, debug=False,
                   num_devices=N_CORES)
    qT = nc.dram_tensor("qT", [D, S], DTR, kind="ExternalInput").ap()
    kT = nc.dram_tensor("kT", [D, S], DTR, kind="ExternalInput").ap()
    vT = nc.dram_tensor("vT", [D, S], DTR, kind="ExternalInput").ap()
    wq = nc.dram_tensor("wq", [D, EC], DTR, kind="ExternalInput").ap()
    wk = nc.dram_tensor("wk", [D, EC], DTR, kind="ExternalInput").ap()
    wv = nc.dram_tensor("wv", [D, EC], DTR, kind="ExternalInput").ap()
    wo = nc.dram_tensor("wo", [EC, D], DTR, kind="ExternalInput").ap()
    ident = nc.dram_tensor("ident", [128, 128], DT, kind="ExternalInput").ap()
    nones = nc.dram_tensor("nones", [1, 128], DTR, kind="ExternalInput").ap()
    attn = nc.dram_tensor("attn", [HC, S, S], DT, kind="ExternalOutput").ap()
    outp = nc.dram_tensor("outp", [S, D], DT, kind="ExternalOutput").ap()

    qTr = qT.rearrange("(n p) s -> n p s", p=128)   # [16,128,S]
    kTr = kT.rearrange("(n p) s -> n p s", p=128)
    vTr = vT.rearrange("(n p) s -> n p s", p=128)
    wqr = wq.rearrange("(n p) e -> n p e", p=128)   # [16,128,512]
    wkr = wk.rearrange("(n p) e -> n p e", p=128)
    wvr = wv.rearrange("(n p) e -> n p e", p=128)
    wor = wo.rearrange("(n p) d -> n p d", p=128)   # [4,128,D]

    with tile.TileContext(nc) as tc:
        with tc.tile_pool(name="big", bufs=14) as big, \
             tc.tile_pool(name="w512", bufs=20) as w512, \
             tc.tile_pool(name="instream", bufs=8) as instream, \
             tc.tile_pool(name="small", bufs=8) as small, \
             tc.tile_pool(name="cpool", bufs=1) as cpool, \
             tc.tile_pool(name="psA", bufs=4, space="PSUM") as psA, \
             tc.tile_pool(name="ps1", bufs=1, space="PSUM") as ps1, \
             tc.tile_pool(name="ps2", bufs=2, space="PSUM") as ps2, \
             tc.tile_pool(name="psO", bufs=1, space="PSUM") as psO:

            def proj_stage(w_tiles, src_r, dst, dst_of):
                """dst[j][:, block] = sum_k lhsT(k,j) @ rhs(k) per s-chunk.
                dst_of(sc, j) -> (tile, col_offset); lhsT/rhs roles differ
                for QT/KT (weights stationary) vs V (acts stationary)."""
                pass  # structured inline below instead

            def body():
                id_sb = cpool.tile([128, 128], DT, tag="ident", name="id_sb")
                nc.sync.dma_start(id_sb[:], ident)
                no_sb = cpool.tile([1, 128], DTR, tag="nones", name="no_sb")
                nc.sync.dma_start(no_sb[:], nones)
                rows = cpool.tile([1, HC * S], DTR, tag="rows", name="rows")

                qt = [big.tile([128, S], DTR, tag="big", name=f"qt{h}")
                      for h in range(HC)]
                kt = [big.tile([128, S], DTR, tag="big", name=f"kt{h}")
                      for h in range(HC)]
                v4 = [big.tile([128, EC * 4], DTR, tag="big", name=f"v4_{j}")
                      for j in range(NSQ // 4)]
                ot = [big.tile([128, S], DTR, tag="big", name=f"ot{h}")
                      for h in range(HC)]

                # ---- stage A: QT (and B: KT) ----
                for (wr, sr, dst) in ((wqr, qTr, qt), (wkr, kTr, kt)):
                    w_t = []
                    for k in range(16):
                        t = w512.tile([128, EC], DTR, tag="w512", name=f"w{k}")
                        nc.sync.dma_start(t[:], wr[k])
                        w_t.append(t)
                    for sc in range(NSC):
                        accs = [psA.tile([128, 512], DT, tag="psA",
                                         name=f"accA{h}") for h in range(HC)]
                        for k in range(16):
                            st = instream.tile([128, 512], DTR,
                                               tag="instream", name="instr")
                            nc.sync.dma_start(
                                st[:], sr[k][:, sc * 512:(sc + 1) * 512])
                            for h in range(HC):
                                nc.tensor.matmul(
                                    accs[h][:],
                                    w_t[k][:, h * 128:(h + 1) * 128],
                                    st[:], start=(k == 0), stop=(k == 15))
                        for h in range(HC):
                            nc.vector.tensor_copy(
                                dst[h][:, sc * 512:(sc + 1) * 512],
                                accs[h][:])

                # ---- stage C: V ----
                wv_t = []
                for k in range(16):
                    t = w512.tile([128, EC], DTR, tag="w512", name=f"wvt{k}")
                    nc.sync.dma_start(t[:], wvr[k])
                    wv_t.append(t)
                for sg in range(NSC):
                    accs = [psA.tile([128, 512], DT, tag="psA",
                                     name=f"accC{t4}") for t4 in range(4)]
                    for k in range(16):
                        st = instream.tile([128, 512], DTR, tag="instream",
                                           name="instr")
                        nc.sync.dma_start(
                            st[:], vTr[k][:, sg * 512:(sg + 1) * 512])
                        for t4 in range(4):
                            nc.tensor.matmul(
                                accs[t4][:],
                                st[:, t4 * 128:(t4 + 1) * 128],
                                wv_t[k][:], start=(k == 0), stop=(k == 15))
                    for t4 in range(4):
                        nc.vector.tensor_copy(
                            v4[sg][:, t4 * EC:(t4 + 1) * EC], accs[t4][:])

                # ---- stages D (pass1) and E (pass2), per head ----
                for h in range(HC):
                    for sq in range(NSQ):
                        ps4 = small.tile([128, NSC], DT, tag="ps4",
                                         name="ps4")
                        echs = []
                        for ck in range(NSC):
                            p = ps1.tile([128, 512], DT, tag="ps1", name="p1")
                            nc.tensor.matmul(
                                p[:], qt[h][:, sq * 128:(sq + 1) * 128],
                                kt[h][:, ck * 512:(ck + 1) * 512],
                                start=True, stop=True)
                            e = w512.tile([128, 512], DT, tag="w512",
                                          name="ech")
                            nc.scalar.activation(
                                e[:], p[:], AF.Exp,
                                accum_out=ps4[:, ck:ck + 1])
                            echs.append(e)
                        tot = small.tile([128, 1], DT, tag="tot", name="tot")
                        nc.vector.reduce_sum(tot[:], ps4[:], axis=AX.X)
                        rec = small.tile([128, 1], DT, tag="rec", name="rec")
                        nc.vector.reciprocal(rec[:], tot[:])
                        lns = small.tile([128, 1], DT, tag="lns", name="lns")
                        nc.scalar.activation(lns[:], tot[:], AF.Ln)
                        pt = ps2.tile([128, 512], DT, tag="ps2", name="ptr")
                        nc.tensor.transpose(pt[:1, :128], lns[:], id_sb[:])
                        nc.scalar.activation(
                            rows[0:1, h * S + sq * 128: h * S + (sq + 1) * 128],
                            pt[:1, :128], AF.Copy)
                        for ck in range(NSC):
                            a = w512.tile([128, 512], DT, tag="w512",
                                          name="attst")
                            nc.vector.tensor_scalar_mul(a[:], echs[ck][:],
                                                        rec[:])
                            nc.sync.dma_start(
                                attn[h, sq * 128:(sq + 1) * 128,
                                     ck * 512:(ck + 1) * 512], a[:])
                    for qc in range(NSC):
                        po = psO.tile([128, 512], DT, tag="psO", name="po")
                        for sk in range(NSQ):
                            p2 = ps2.tile([128, 512], DT, tag="ps2",
                                          name="p2")
                            nc.tensor.matmul(
                                p2[:], kt[h][:, sk * 128:(sk + 1) * 128],
                                qt[h][:, qc * 512:(qc + 1) * 512],
                                start=True, stop=False)
                            nc.tensor.matmul(
                                p2[:], no_sb[:],
                                rows[0:1, h * S + qc * 512:
                                     h * S + (qc + 1) * 512],
                                start=False, stop=True)
                            et = w512.tile([128, 512], DTR, tag="w512",
                                           name="et")
                            nc.scalar.activation(et[:], p2[:], AF.Exp)
                            nc.tensor.matmul(
                                po[:],
                                v4[sk // 4][:, (sk % 4) * EC + h * 128:
                                            (sk % 4) * EC + (h + 1) * 128],
                                et[:], start=(sk == 0), stop=(sk == NSQ - 1))
                        nc.vector.tensor_copy(
                            ot[h][:, qc * 512:(qc + 1) * 512], po[:])

                # ---- stage F: O-projection ----
                wo_t = []
                for j in range(HC):
                    t = big.tile([128, D], DTR, tag="big", name=f"wo{j}")
                    nc.sync.dma_start(t[:], wor[j])
                    wo_t.append(t)
                for dc in range(D // 512):
                    for sq in range(NSQ):
                        pf = psA.tile([128, 512], DT, tag="psA", name="pf")
                        for h in range(HC):
                            nc.tensor.matmul(
                                pf[:], ot[h][:, sq * 128:(sq + 1) * 128],
                                wo_t[h][:, dc * 512:(dc + 1) * 512],
                                start=(h == 0), stop=(h == HC - 1))
                        o = w512.tile([128, 512], DT, tag="w512", name="ost")
                        nc.vector.tensor_copy(o[:], pf[:])
                        nc.sync.dma_start(
                            outp[sq * 128:(sq + 1) * 128,
                                 dc * 512:(dc + 1) * 512], o[:])

            if reps == 1:
                body()
            else:
                with tc.For_i(0, reps, 1):
                    body()
    nc.compile()
    return nc


def make_in_maps(query, key, value, W_q, W_kc, W_vc, W_ku, W_vu, W_o):
    query = np.asarray(query, np.float32)
    key = np.asarray(key, np.float32)
    value = np.asarray(value, np.float32)
    W_q = np.asarray(W_q, np.float32)
    W_kc = np.asarray(W_kc, np.float32)
    W_vc = np.asarray(W_vc, np.float32)
    W_ku = np.asarray(W_ku, np.float32)
    W_vu = np.asarray(W_vu, np.float32)
    W_o = np.asarray(W_o, np.float32)

    scale = np.float32(1.0 / np.sqrt(DK))
    Wq = round_fp32r(W_q * scale)
    Wk = round_fp32r((W_kc.astype(np.float64) @ W_ku.astype(np.float64))
                     .astype(np.float32))
    Wv = round_fp32r((W_vc.astype(np.float64) @ W_vu.astype(np.float64))
                     .astype(np.float32))
    Wo = round_fp32r(W_o)
    qT = [round_fp32r(query[b].T) for b in range(B)]
    kT = [round_fp32r(key[b].T) for b in range(B)]
    vT = [round_fp32r(value[b].T) for b in range(B)]
    ident = np.eye(128, dtype=np.float32)
    nones = np.full((1, 128), -1.0, np.float32)

    in_maps = []
    for c in range(N_CORES):
        b, g = divmod(c, B * 2)
        in_maps.append({
            "qT": qT[b], "kT": kT[b], "vT": vT[b],
            "wq": np.ascontiguousarray(Wq[:, g * EC:(g + 1) * EC]),
            "wk": np.ascontiguousarray(Wk[:, g * EC:(g + 1) * EC]),
            "wv": np.ascontiguousarray(Wv[:, g * EC:(g + 1) * EC]),
            "wo": np.ascontiguousarray(Wo[g * EC:(g + 1) * EC, :]),
            "ident": ident, "nones": nones,
        })
    return in_maps


def kernel(query, key, value, W_q, W_kc, W_vc, W_ku, W_vu, W_o):
    in_maps = make_in_maps(query, key, value, W_q, W_kc, W_vc, W_ku,
                           W_vu, W_o)
    nc = build_nc(reps=1)
    br = run_bass_kernel_spmd(nc, in_maps, list(range(N_CORES)))
    attn = np.empty((B, H, S_FULL, S_FULL), np.float32)
    output = np.zeros((B, S_FULL, D), np.float32)
    for c in range(N_CORES):
        b, g = divmod(c, B * 2)
        r = br.results[c]
        attn[b, g * HC:(g + 1) * HC] = r["attn"].reshape(HC, S_FULL, S_FULL)
        output[b] += r["outp"].reshape(S_FULL, D)
    return output, attn


# revision 1
# speedup vs baseline: 5.7462x; 5.7462x over previous
"""Multi-Head Latent Attention on 8 trn2 NeuronCores (Bass/Tile).

Sharding: core c = 4*b + g handles batch b, head group g (4 of 16 heads).
Host absorbs the latent down/up projections (W_k = W_kc @ W_ku etc.), folds
the 1/sqrt(d_k) scale into W_q, pre-transposes q/k/v to [D, S], and
pre-rounds all matmul inputs to fp32r (the PE's fast 4-byte format; exact
on pre-rounded data).

Per-core device pipeline (S=2048, D=2048, d_k=128, 4 heads, e=4*128=512):
  A. QT[h] [128,2048] = (W_q_h.T q.T)      (d-outer accumulate, N=512)
  B. KT[h] [128,2048] = (W_k_h.T k.T)
  C. V    [S,512] as 4 tiles [128,2048]    (v.T stationary, W_v moving)
  D. pass1 per (h, sq-tile): scores = QT_h.T KT_h -> exp (accum row sums)
     -> attn = E * (1/sum) -> HBM; also ln(sum) transposed into a row
  E. pass2 per (h, sq-chunk): scoresT = KT_h.T QT_h; add -ln(sum) row via
     K=1 ones-matmul; exp -> normalized attn^T (fp32r); AV accumulate
     outT[h] [128 e, 2048 sq]
  F. O-proj: out[sq, dm] = sum_h OT_h.T W_o_h -> HBM (normalized already)
Host: attn = stack of per-core [4,S,S]; output[b] = sum of 4 cores' outp.
"""
import numpy as np
import concourse.bass as bass
import concourse.mybir as mybir
from concourse import bacc, tile
from concourse.bass_utils import run_bass_kernel_spmd

AF = mybir.ActivationFunctionType
DT = mybir.dt.float32
DTR = mybir.dt.float32r
AX = mybir.AxisListType

B, S_FULL, D, H, DK, L = 2, 2048, 2048, 16, 128, 512
N_CORES = 8
HC = 4          # heads per core
EC = HC * DK    # 512


def round_fp32r(x):
    x = np.ascontiguousarray(x, dtype=np.float32)
    u = x.view(np.uint32).astype(np.uint64)
    u = (u + 0x7FF + ((u >> 12) & 1)) & 0xFFFFF000
    return u.astype(np.uint32).view(np.float32)


def build_nc(reps=1, S=S_FULL):
    NSQ = S // 128   # sq tiles
    NSC = S // 512   # 512-wide chunks
    nc = bacc.Bacc("TRN2", target_bir_lowering=False, debug=False,
                   num_devices=N_CORES)
    qT = nc.dram_tensor("qT", [D, S], DTR, kind="ExternalInput").ap()
    kT = nc.dram_tensor("kT", [D, S], DTR, kind="ExternalInput").ap()
    vT = nc.dram_tensor("vT", [D, S], DTR, kind="ExternalInput").ap()
    wq = nc.dram_tensor("wq", [D, EC], DTR, kind="ExternalInput").ap()
    wk = nc.dram_tensor("wk", [D, EC], DTR, kind="ExternalInput").ap()
    wv = nc.dram_tensor("wv", [D, EC], DTR, kind="ExternalInput").ap()
    wo = nc.dram_tensor("wo", [EC, D], DTR, kind="ExternalInput").ap()
    ident = nc.dram_tensor("ident", [128, 128], DT, kind="ExternalInput").ap()
    nones = nc.dram_tensor("nones", [1, 128], DTR, kind="ExternalInput").ap()
    attn = nc.dram_tensor("attn", [HC, S, S], DT, kind="ExternalOutput").ap()
    outp = nc.dram_tensor("outp", [S, D], DT, kind="ExternalOutput").ap()

    qTr = qT.rearrange("(n p) s -> n p s", p=128)   # [16,128,S]
    kTr = kT.rearrange("(n p) s -> n p s", p=128)
    vTr = vT.rearrange("(n p) s -> n p s", p=128)
    wqr = wq.rearrange("(n p) e -> n p e", p=128)   # [16,128,512]
    wkr = wk.rearrange("(n p) e -> n p e", p=128)
    wvr = wv.rearrange("(n p) e -> n p e", p=128)
    wor = wo.rearrange("(n p) d -> n p d", p=128)   # [4,128,D]

    with tile.TileContext(nc) as tc:
        with tc.tile_pool(name="big", bufs=14) as big, \
             tc.tile_pool(name="w512", bufs=20) as w512, \
             tc.tile_pool(name="instream", bufs=8) as instream, \
             tc.tile_pool(name="small", bufs=8) as small, \
             tc.tile_pool(name="cpool", bufs=1) as cpool, \
             tc.tile_pool(name="psA", bufs=4, space="PSUM") as psA, \
             tc.tile_pool(name="ps1", bufs=1, space="PSUM") as ps1, \
             tc.tile_pool(name="ps2", bufs=2, space="PSUM") as ps2, \
             tc.tile_pool(name="psO", bufs=1, space="PSUM") as psO:

            def proj_stage(w_tiles, src_r, dst, dst_of):
                """dst[j][:, block] = sum_k lhsT(k,j) @ rhs(k) per s-chunk.
                dst_of(sc, j) -> (tile, col_offset); lhsT/rhs roles differ
                for QT/KT (weights stationary) vs V (acts stationary)."""
                pass  # structured inline below instead

            def body():
                id_sb = cpool.tile([128, 128], DT, tag="ident", name="id_sb")
                nc.sync.dma_start(id_sb[:], ident)
                no_sb = cpool.tile([1, 128], DTR, tag="nones", name="no_sb")
                nc.sync.dma_start(no_sb[:], nones)
                rows = cpool.tile([1, HC * S], DTR, tag="rows", name="rows")

                qt = [big.tile([128, S], DTR, tag="big", name=f"qt{h}")
                      for h in range(HC)]
                kt = [big.tile([128, S], DTR, tag="big", name=f"kt{h}")
                      for h in range(HC)]
                v4 = [big.tile([128, EC * 4], DTR, tag="big", name=f"v4_{j}")
                      for j in range(NSQ // 4)]
                ot = [big.tile([128, S], DTR, tag="big", name=f"ot{h}")
                      for h in range(HC)]

                # ---- stage A: QT (and B: KT) ----
                for (wr, sr, dst) in ((wqr, qTr, qt), (wkr, kTr, kt)):
                    w_t = []
                    for k in range(16):
                        t = w512.tile([128, EC], DTR, tag="w512", name=f"w{k}")
                        nc.sync.dma_start(t[:], wr[k])
                        w_t.append(t)
                    for sc in range(NSC):
                        accs = [psA.tile([128, 512], DT, tag="psA",
                                         name=f"accA{h}") for h in range(HC)]
                        for k in range(16):
                            st = instream.tile([128, 512], DTR,
                                               tag="instream", name="instr")
                            nc.sync.dma_start(
                                st[:], sr[k][:, sc * 512:(sc + 1) * 512])
                            for h in range(HC):
                                nc.tensor.matmul(
                                    accs[h][:],
                                    w_t[k][:, h * 128:(h + 1) * 128],
                                    st[:], start=(k == 0), stop=(k == 15))
                        for h in range(HC):
                            nc.vector.tensor_copy(
                                dst[h][:, sc * 512:(sc + 1) * 512],
                                accs[h][:])

                # ---- stage C: V ----
                wv_t = []
                for k in range(16):
                    t = w512.tile([128, EC], DTR, tag="w512", name=f"wvt{k}")
                    nc.sync.dma_start(t[:], wvr[k])
                    wv_t.append(t)
                for sg in range(NSC):
                    accs = [psA.tile([128, 512], DT, tag="psA",
                                     name=f"accC{t4}") for t4 in range(4)]
                    for k in range(16):
                        st = instream.tile([128, 512], DTR, tag="instream",
                                           name="instr")
                        nc.sync.dma_start(
                            st[:], vTr[k][:, sg * 512:(sg + 1) * 512])
                        for t4 in range(4):
                            nc.tensor.matmul(
                                accs[t4][:],
                                st[:, t4 * 128:(t4 + 1) * 128],
                                wv_t[k][:], start=(k == 0), stop=(k == 15))
                    for t4 in range(4):
                        nc.vector.tensor_copy(
                            v4[sg][:, t4 * EC:(t4 + 1) * EC], accs[t4][:])

                # ---- stages D (pass1) and E (pass2), per head ----
                for h in range(HC):
                    for sq in range(NSQ):
                        ps4 = small.tile([128, NSC], DT, tag="ps4",
                                         name="ps4")
                        echs = []
                        for ck in range(NSC):
                            p = ps1.tile([128, 512], DT, tag="ps1", name="p1")
                            nc.tensor.matmul(
                                p[:], qt[h][:, sq * 128:(sq + 1) * 128],
                                kt[h][:, ck * 512:(ck + 1) * 512],
                                start=True, stop=True)
                            e = w512.tile([128, 512], DT, tag="w512",
                                          name="ech")
                            nc.scalar.activation(
                                e[:], p[:], AF.Exp,
                                accum_out=ps4[:, ck:ck + 1])
                            echs.append(e)
                        tot = small.tile([128, 1], DT, tag="tot", name="tot")
                        nc.vector.reduce_sum(tot[:], ps4[:], axis=AX.X)
                        rec = small.tile([128, 1], DT, tag="rec", name="rec")
                        nc.vector.reciprocal(rec[:], tot[:])
                        lns = small.tile([128, 1], DT, tag="lns", name="lns")
                        nc.scalar.activation(lns[:], tot[:], AF.Ln)
                        pt = ps2.tile([128, 512], DT, tag="ps2", name="ptr")
                        nc.tensor.transpose(pt[:1, :128], lns[:], id_sb[:])
                        nc.scalar.activation(
                            rows[0:1, h * S + sq * 128: h * S + (sq + 1) * 128],
                            pt[:1, :128], AF.Copy)
                        for ck in range(NSC):
                            a = w512.tile([128, 512], DT, tag="w512",
                                          name="attst")
                            nc.vector.tensor_scalar_mul(a[:], echs[ck][:],
                                                        rec[:])
                            nc.sync.dma_start(
                                attn[h, sq * 128:(sq + 1) * 128,
                                     ck * 512:(ck + 1) * 512], a[:])
                    for qc in range(NSC):
                        po = psO.tile([128, 512], DT, tag="psO", name="po")
                        for sk in range(NSQ):
                            p2 = ps2.tile([128, 512], DT, tag="ps2",
                                          name="p2")
                            nc.tensor.matmul(
                                p2[:], kt[h][:, sk * 128:(sk + 1) * 128],
                                qt[h][:, qc * 512:(qc + 1) * 512],
                                start=True, stop=False)
                            nc.tensor.matmul(
                                p2[:], no_sb[:],
                                rows[0:1, h * S + qc * 512:
                                     h * S + (qc + 1) * 512],
                                start=False, stop=True)
                            et = w512.tile([128, 512], DTR, tag="w512",
                                           name="et")
                            nc.scalar.activation(et[:], p2[:], AF.Exp)
                            nc.tensor.matmul(
                                po[:],
                                v4[sk // 4][:, (sk % 4) * EC + h * 128:
                                            (sk % 4) * EC + (h + 1) * 128],
                                et[:], start=(sk == 0), stop=(sk == NSQ - 1))
                        nc.vector.tensor_copy(
                            ot[h][:, qc * 512:(qc + 1) * 512], po[:])

                # ---- stage F: O-projection ----
                wo_t = []
                for j in range(HC):
                    t = big.tile([128, D], DTR, tag="big", name=f"wo{j}")
                    nc.sync.dma_start(t[:], wor[j])
                    wo_t.append(t)
                for dc in range(D // 512):
                    for sq in range(NSQ):
                        pf = psA.tile([128, 512], DT, tag="psA", name="pf")
                        for h in range(HC):
                            nc.tensor.matmul(
                                pf[:], ot[h][:, sq * 128:(sq + 1) * 128],
                                wo_t[h][:, dc * 512:(dc + 1) * 512],
                                start=(h == 0), stop=(h == HC - 1))
                        o = w512.tile([128, 512], DT, tag="w512", name="ost")
                        nc.vector.tensor_copy(o[:], pf[:])
                        nc.sync.dma_start(
                            outp[sq * 128:(sq + 1) * 128,
                                 dc * 512:(dc + 1) * 512], o[:])

            if reps == 1:
                body()
            else:
                with tc.For_i(0, reps, 1):
                    body()
    nc.compile()
    return nc


def make_in_maps(query, key, value, W_q, W_kc, W_vc, W_ku, W_vu, W_o):
    query = np.asarray(query, np.float32)
    key = np.asarray(key, np.float32)
    value = np.asarray(value, np.float32)
    W_q = np.asarray(W_q, np.float32)
    W_kc = np.asarray(W_kc, np.float32)
    W_vc = np.asarray(W_vc, np.float32)
    W_ku = np.asarray(W_ku, np.float32)
    W_vu = np.asarray(W_vu, np.float32)
    W_o = np.asarray(W_o, np.float32)

    scale = np.float32(1.0 / np.sqrt(DK))
    Wq = round_fp32r(W_q * scale)
    Wk = round_fp32r((W_kc.astype(np.float64) @ W_ku.astype(np.float64))
                     .astype(np.float32))
    Wv = round_fp32r((W_vc.astype(np.float64) @ W_vu.astype(np.float64))
                     .astype(np.float32))
    Wo = round_fp32r(W_o)
    qT = [round_fp32r(query[b].T) for b in range(B)]
    kT = [round_fp32r(key[b].T) for b in range(B)]
    vT = [round_fp32r(value[b].T) for b in range(B)]
    ident = np.eye(128, dtype=np.float32)
    nones = np.full((1, 128), -1.0, np.float32)

    in_maps = []
    for c in range(N_CORES):
        b, g = divmod(c, B * 2)
        in_maps.append({
            "qT": qT[b], "kT": kT[b], "vT": vT[b],
            "wq": np.ascontiguousarray(Wq[:, g * EC:(g + 1) * EC]),
            "wk": np.ascontiguousarray(Wk[:, g * EC:(g + 1) * EC]),
            "wv": np.ascontiguousarray(Wv[:, g * EC:(g + 1) * EC]),
            "wo": np.ascontiguousarray(Wo[g * EC:(g + 1) * EC, :]),
            "ident": ident, "nones": nones,
        })
    return in_maps


def kernel(query, key, value, W_q, W_kc, W_vc, W_ku, W_vu, W_o):
    in_maps = make_in_maps(query, key, value, W_q, W_kc, W_vc, W_ku,
                           W_vu, W_o)
    nc = build_nc(reps=1)
    br = run_bass_kernel_spmd(nc, in_maps, list(range(N_CORES)))
    attn = np.empty((B, H, S_FULL, S_FULL), np.float32)
    output = np.zeros((B, S_FULL, D), np.float32)
    for c in range(N_CORES):
        b, g = divmod(c, B * 2)
        r = br.results[c]
        attn[b, g * HC:(g + 1) * HC] = r["attn"].reshape(HC, S_FULL, S_FULL)
        output[b] += r["outp"].reshape(S_FULL, D)
    return output, attn
